# revision 13
# baseline (speedup 1.0000x reference)
"""Causal self-attention (B=4, T=2048, C=1024, 16 heads, fp32) on 8 TRN2 NeuronCores.

Sharding: 8 cores = 4 batches x 2 head-groups (8 heads each); identical program
per core.  bf16 on-device compute (inputs pre-cast on host, DMA'd straight into
matmul-ready tiles — no SBUF dtype-staging copies), fp32 PSUM accumulation.

  phase 1: QKV projection in 512-token chunks (chunk == attention q-chunk).
           Q^T/K^T in [head*64+d, T] pair-tile layout, V in [T, head*64+d]
           augmented with a ones column per head (AV matmul then also emits the
           softmax denominators for free).
  phase 2: flash-style causal attention per head pair.  S^T[k,q] via two
           64-contract matmuls per key block placed on disjoint PE row groups
           (0/64 -> concurrent tiles), one exp per key block on ScalarE out of
           PSUM into bf16 P, diagonal blocks masked by a triangular bf16
           tensor_mul; diagonal S/AV matmuls narrowed to the exact unmasked
           q-range (bf16 has no fp32r >=256 moving-width constraint).  O^T and
           denominators accumulate in PSUM; normalization on PSUM->SBUF
           copy-out (reciprocal + partition_broadcast + tensor_mul) writes y^T
           over the dead Q^T chunk.
  phase 3: output projection, partial [T, C] per core, summed on host.

Scheduling: attention is exp-rate-bound on ScalarE (~1us/key-block vs ~0.64us
of TensorE work), so phase-1/proj matmul units are interleaved between
attention key-block units at computed ratios to keep TensorE saturated;
attention for q-chunk qc starts as soon as chunk qc's QKV is done.

Host side: per-batch pairs of partial outputs summed (2-way all-reduce of the
row-sharded Wproj) plus the rank-1 correction (bqkv_v @ Wproj + bproj), which
commutes with attention because softmax rows sum to one.  Softmax
max-subtraction skipped: scores ~N(0,1) after the 1/8 scale, exp cannot
overflow.
"""
import itertools

import numpy as np
import ml_dtypes

import concourse.bass as bass  # noqa: F401  (bass must be imported before tile)
import concourse.tile as tile
from concourse import mybir
from concourse.bacc import Bacc
from concourse.bass_utils import run_bass_kernel_spmd

F32 = mybir.dt.float32
BF16 = mybir.dt.bfloat16
BF = ml_dtypes.bfloat16

B, T, C = 4, 2048, 1024
NH = 16          # total heads
D = 64           # head dim
G = 2            # head groups (cores per batch)
HPG = NH // G    # heads per group = 8
GC = HPG * D     # columns per group = 512
CT = C // 128    # contraction tiles = 8
QCW = 512        # chunk width (phase-1 chunk == attention q-chunk)
NQC = T // QCW   # 4 q-chunks
NTT = T // 128   # 16 t-tiles
NHP = HPG // 2   # head pairs per core = 4
EXP = mybir.ActivationFunctionType.Exp

# When True, S scores land in PSUM as bf16 (ph0 in bank 0 cols [0:QCW],
# ph1 in bank 1 cols [2*QCW:3*QCW]) and exp runs as two narrowed bf16-in
# instructions per key block instead of one merged fp32-in [128,1024].
S_BF16 = False

_SENT = object()


def build(reps=1):
    nc = Bacc()
    xT = nc.dram_tensor("xT", [C, T], BF16, kind="ExternalInput")
    wqk = nc.dram_tensor("wqk", [C, 2 * GC], BF16, kind="ExternalInput")
    wv = nc.dram_tensor("wv", [C, GC], BF16, kind="ExternalInput")
    wp = nc.dram_tensor("wp", [GC, C], BF16, kind="ExternalInput")
    bqk = nc.dram_tensor("bqk", [128, 2 * GC // 128], F32, kind="ExternalInput")
    out = nc.dram_tensor("out", [T, C], F32, kind="ExternalOutput")

    with tile.TileContext(nc) as tc:
        with (
            tc.tile_pool(name="persist", bufs=1) as pp,
            tc.tile_pool(name="xc", bufs=2) as xcp,
            tc.tile_pool(name="pt", bufs=5) as ptp,
            tc.tile_pool(name="rb", bufs=3) as rbp,
            tc.tile_pool(name="ost", bufs=3) as ost,
            tc.tile_pool(name="ps", bufs=2, space="PSUM") as ps,
            tc.tile_pool(name="psS", bufs=2, space="PSUM") as psS,
            tc.tile_pool(name="psO", bufs=1, space="PSUM") as psO,
        ):
            # long-lived SBUF tensors.  QT[j][qc] doubles as y^T storage.
            # For timing builds (reps>1) two alternating sets decouple the
            # rep seams (rep r+1's QKV writes would otherwise wait on rep r's
            # late attention/proj readers, serializing the stream).
            def qkv_set(s):
                QT = [[pp.tile([128, QCW], BF16, tag=f"qt{s}_{j}_{q}",
                               name=f"qt{s}_{j}_{q}")
                       for q in range(NQC)] for j in range(NHP)]
                KT = [[pp.tile([128, QCW], BF16, tag=f"kt{s}_{j}_{q}",
                               name=f"kt{s}_{j}_{q}")
                       for q in range(NQC)] for j in range(NHP)]
                VA = [pp.tile([128, HPG, D + 1], BF16, tag=f"va{s}_{t}",
                              name=f"va{s}_{t}")
                      for t in range(NTT)]
                return QT, KT, VA

            sets = [qkv_set(0)]
            if reps > 1:
                sets.append(qkv_set(1))
            QT, KT, VA = sets[0]
            YT = QT
            WQK = [pp.tile([128, 2 * GC], BF16, tag=f"wqk{c}", name=f"wqk{c}")
                   for c in range(CT)]
            WV = [pp.tile([128, GC], BF16, tag=f"wv{c}", name=f"wv{c}")
                  for c in range(CT)]
            WP = [pp.tile([128, C], BF16, tag=f"wpr{j}", name=f"wpr{j}")
                  for j in range(GC // 128)]
            bqk_sb = pp.tile([128, 2 * GC // 128], F32)

            # weight / bias loads (DMA straight into bf16 tiles)
            for c in range(CT):
                nc.sync.dma_start(out=WQK[c], in_=wqk[128 * c:128 * (c + 1), :])
            nc.sync.dma_start(out=bqk_sb, in_=bqk[:])
            for c in range(CT):
                nc.sync.dma_start(out=WV[c], in_=wv[128 * c:128 * (c + 1), :])

            # upper-triangular (keep k<=q) bf16 mask for diagonal sub-blocks
            tri32 = pp.tile([128, 128], F32)
            nc.vector.memset(tri32, 1.0)
            nc.gpsimd.affine_select(
                out=tri32, in_=tri32, pattern=[[1, 128]],
                compare_op=mybir.AluOpType.is_ge, fill=0.0,
                base=0, channel_multiplier=-1,
            )
            tri = pp.tile([128, 128], BF16)
            nc.vector.tensor_copy(tri, tri32)
            # ones columns of VA (disjoint from the V slices written later)
            for _, _, va_s in sets:
                for t in range(NTT):
                    nc.gpsimd.memset(va_s[t][:, :, D:D + 1], 1.0)

            def p1(ch):
                XC = []
                for c in range(CT):
                    xc = xcp.tile([128, QCW], BF16, tag=f"xc{c}", name=f"xc{c}")
                    nc.sync.dma_start(
                        out=xc, in_=xT[128 * c:128 * (c + 1), QCW * ch:QCW * (ch + 1)]
                    )
                    XC.append(xc)
                yield
                # Q^T / K^T row-tiles (m<4 -> Q pair-tile m, m>=4 -> K pair m-4)
                # split into half-units (4 MMs each) so interleave filler is
                # ~0.55us-granular; the open accumulation group across the
                # yield is fine (other tiles' matmuls don't touch this bank)
                for m in range(2 * GC // 128):
                    acc = ps.tile([128, QCW], F32, tag="pp", name="pp")
                    for c in range(CT // 2):
                        nc.tensor.matmul(
                            acc, WQK[c][:, 128 * m:128 * (m + 1)], XC[c],
                            start=(c == 0), stop=False,
                        )
                    yield
                    for c in range(CT // 2, CT):
                        nc.tensor.matmul(
                            acc, WQK[c][:, 128 * m:128 * (m + 1)], XC[c],
                            start=False, stop=(c == CT - 1),
                        )
                    dst = QT[m][ch] if m < NHP else KT[m - NHP][ch]
                    nc.vector.tensor_scalar_add(dst, acc, bqk_sb[:, m:m + 1])
                    yield
                for ti in range(QCW // 128):
                    t = (QCW // 128) * ch + ti
                    acc = ps.tile([128, QCW], F32, tag="pp", name="pp")
                    for c in range(CT // 2):
                        nc.tensor.matmul(
                            acc, XC[c][:, 128 * ti:128 * (ti + 1)], WV[c],
                            start=(c == 0), stop=False,
                        )
                    yield
                    for c in range(CT // 2, CT):
                        nc.tensor.matmul(
                            acc, XC[c][:, 128 * ti:128 * (ti + 1)], WV[c],
                            start=False, stop=(c == CT - 1),
                        )
                    nc.vector.tensor_copy(
                        VA[t][:, :, 0:D], acc.rearrange("p (h d) -> p h d", h=HPG)
                    )
                    yield

            N_P1 = 1 + 16 + 8

            def att(qc):
                # Software-pipelined: S/exp/mask for kb+1 are emitted BEFORE
                # AV for kb, and the interleave filler (yield) lands between
                # them — so TensorE's in-order queue covers the exp latency
                # with filler matmuls instead of blocking on AV, and ScalarE's
                # exp stream (fed by the already-issued S) never starves.
                kbmax = 4 * (qc + 1)

                def s_exp(hp, kb):
                    j = kb - 4 * qc
                    s_off = 128 * j if j > 0 else 0
                    P = ptp.tile([128, 2 * QCW], BF16, tag="p", name="p")
                    if S_BF16:
                        # ph0 -> bank 0, ph1 -> bank 1 (keeps the concurrent
                        # 0/64 row-group pair off the same PSUM bank)
                        S = psS.tile([128, 4 * QCW], BF16, tag="s", name="s")
                        for ph in range(2):
                            p_sl = slice(64 * ph, 64 * (ph + 1))
                            nc.tensor.matmul(
                                S[:, 2 * QCW * ph + s_off:2 * QCW * ph + QCW],
                                KT[hp][kb // 4][p_sl, 128 * (kb % 4):128 * (kb % 4 + 1)],
                                QT[hp][qc][p_sl, s_off:],
                                start=True, stop=True,
                            )
                        for ph in range(2):
                            nc.scalar.activation(
                                out=P[:, QCW * ph + s_off:QCW * (ph + 1)],
                                in_=S[:, 2 * QCW * ph + s_off:2 * QCW * ph + QCW],
                                func=EXP, scale=0.125,
                            )
                    else:
                        S = psS.tile([128, 2 * QCW], F32, tag="s", name="s")
                        for ph in range(2):
                            p_sl = slice(64 * ph, 64 * (ph + 1))
                            nc.tensor.matmul(
                                S[:, QCW * ph + s_off:QCW * (ph + 1)],
                                KT[hp][kb // 4][p_sl, 128 * (kb % 4):128 * (kb % 4 + 1)],
                                QT[hp][qc][p_sl, s_off:],
                                start=True, stop=True,
                            )
                        nc.scalar.activation(out=P, in_=S, func=EXP, scale=0.125)
                    if j >= 0:
                        # masks on GpSimd (idle engine, SBUF-only op): keeps
                        # the exp->mask->AV chain off the busy DVE queue
                        for ph in range(2):
                            off = QCW * ph + 128 * j
                            nc.gpsimd.tensor_mul(
                                P[:, off:off + 128], P[:, off:off + 128], tri
                            )
                    return P, s_off

                def norm(hp, O):
                    rc = rbp.tile([1, 2 * QCW], F32, tag="rc", name="rc", bufs=1)
                    nc.vector.reciprocal(rc, O[D:D + 1, :])
                    for ph in range(2):
                        rb = rbp.tile([64, QCW], F32, tag="rb", name="rb")
                        nc.gpsimd.partition_broadcast(rb, rc[0:1, QCW * ph:QCW * (ph + 1)])
                        nc.vector.tensor_mul(
                            YT[hp][qc][64 * ph:64 * (ph + 1), :],
                            O[0:D, QCW * ph:QCW * (ph + 1)], rb,
                        )

                pend = None  # previous head-pair's (hp, O) awaiting norm
                for hp in range(NHP):
                    O = psO.tile([D + 1, 2 * QCW], F32, tag="o", name="o")
                    pipe = s_exp(hp, 0)
                    if pend is not None:
                        # psO bufs=1: norm(hp-1) MUST be emitted before any
                        # AV of this hp (same PSUM addresses)
                        norm(*pend)
                        pend = None
                        yield
                    for kb in range(kbmax):
                        P, s_off = pipe
                        if kb + 1 < kbmax:
                            pipe = s_exp(hp, kb + 1)
                        yield
                        for ph in range(2):
                            nc.tensor.matmul(
                                O[:, QCW * ph + s_off:QCW * (ph + 1)],
                                VA[kb][:, 2 * hp + ph, :],
                                P[:, QCW * ph + s_off:QCW * (ph + 1)],
                                start=(kb == 0), stop=(kb == kbmax - 1),
                            )
                    pend = (hp, O)
                norm(*pend)
                yield

            def n_att(qc):
                return NHP * (4 * (qc + 1)) + NHP

            def proj(qc):
                for ti in range(4):
                    t = 4 * qc + ti
                    for nn in range(2):
                        acc = ps.tile([128, QCW], F32, tag="pp", name="pp")
                        for jj in range(GC // 128):
                            nc.tensor.matmul(
                                acc,
                                YT[jj][qc][:, 128 * ti:128 * (ti + 1)],
                                WP[jj][:, QCW * nn:QCW * (nn + 1)],
                                start=(jj == 0), stop=(jj == GC // 128 - 1),
                            )
                        o = ost.tile([128, QCW], F32, tag="o", name="o")
                        nc.vector.tensor_copy(o, acc)
                        nc.sync.dma_start(
                            out=out[128 * t:128 * (t + 1), QCW * nn:QCW * (nn + 1)],
                            in_=o,
                        )
                        yield

            def load_wp():
                for jx in range(GC // 128):
                    nc.sync.dma_start(out=WP[jx], in_=wp[128 * jx:128 * (jx + 1), :])
                yield

            def interleave(main_gen, n_main, fill_gen, n_fill):
                i = pulled = 0
                for _ in main_gen:
                    i += 1
                    tgt = i * n_fill // n_main
                    while pulled < tgt and next(fill_gen, _SENT) is not _SENT:
                        pulled += 1
                while next(fill_gen, _SENT) is not _SENT:
                    pass

            for rep in range(reps):
                QT, KT, VA = sets[rep % len(sets)]
                YT = QT
                for _ in p1(0):
                    pass
                interleave(att(0), n_att(0), p1(1), N_P1)
                fill1 = itertools.chain(p1(2), load_wp()) if rep == 0 else p1(2)
                interleave(att(1), n_att(1), fill1, N_P1 + 1)
                interleave(att(2), n_att(2), p1(3), N_P1)
                interleave(
                    att(3), n_att(3),
                    itertools.chain(proj(0), proj(1), proj(2)), 24,
                )
                for _ in proj(3):
                    pass
    nc.finalize()
    return nc


_NC = None


def _get_nc():
    global _NC
    if _NC is None:
        _NC = build()
    return _NC


def _shard(x, Wqkv, bqkv, Wproj):
    in_maps = []
    for core in range(8):
        b, g = core // G, core % G
        cs = slice(GC * g, GC * (g + 1))
        wqk_h = np.concatenate([Wqkv[:, cs], Wqkv[:, C:][:, cs]], axis=1)
        bqk_h = np.concatenate([bqkv[cs], bqkv[C:][cs.start:cs.stop]])
        in_maps.append({
            "xT": np.ascontiguousarray(x[b].T.astype(BF)),
            "wqk": np.ascontiguousarray(wqk_h.astype(BF)),
            "wv": np.ascontiguousarray(Wqkv[:, 2 * C:][:, cs].astype(BF)),
            "wp": np.ascontiguousarray(Wproj[cs, :].astype(BF)),
            "bqk": np.ascontiguousarray(bqk_h.reshape(2 * GC // 128, 128).T),
        })
    return in_maps


def kernel(x, Wqkv, bqkv, Wproj, bproj, _want_results=False, **run_kwargs):
    x = np.ascontiguousarray(np.asarray(x, dtype=np.float32))
    Wqkv = np.ascontiguousarray(np.asarray(Wqkv, dtype=np.float32))
    bqkv = np.ascontiguousarray(np.asarray(bqkv, dtype=np.float32))
    Wproj = np.ascontiguousarray(np.asarray(Wproj, dtype=np.float32))
    bproj = np.ascontiguousarray(np.asarray(bproj, dtype=np.float32))

    nc = _get_nc()
    in_maps = _shard(x, Wqkv, bqkv, Wproj)
    res = run_bass_kernel_spmd(nc, in_maps, core_ids=list(range(8)), **run_kwargs)

    out = np.empty((B, T, C), dtype=np.float32)
    for b in range(B):
        out[b] = res.results[G * b]["out"]
        for g in range(1, G):
            out[b] += res.results[G * b + g]["out"]
    # rank-1 corrections: v-bias (rows of softmax sum to 1) and proj bias
    out += bqkv[2 * C:] @ Wproj + bproj
    if _want_results:
        return out, res
    return out


# revision 14
# speedup vs baseline: 1.2623x; 1.2623x over previous
"""Causal self-attention (B=4, T=2048, C=1024, 16 heads, fp32) on 8 TRN2 NeuronCores.

Sharding: 8 cores = 4 batches x 2 head-groups (8 heads each); identical program
per core.  bf16 on-device compute (inputs pre-cast on host, DMA'd straight into
matmul-ready tiles — no SBUF dtype-staging copies), fp32 PSUM accumulation.

  phase 1: QKV projection in 512-token chunks (chunk == attention q-chunk).
           Q^T/K^T in [head*64+d, T] pair-tile layout, V in [T, head*64+d]
           augmented with a ones column per head (AV matmul then also emits the
           softmax denominators for free).
  phase 2: flash-style causal attention per head pair.  S^T[k,q] via two
           64-contract matmuls per key block placed on disjoint PE row groups
           (0/64 -> concurrent tiles), one exp per key block on ScalarE out of
           PSUM into bf16 P, diagonal blocks masked by a triangular bf16
           tensor_mul; diagonal S/AV matmuls narrowed to the exact unmasked
           q-range (bf16 has no fp32r >=256 moving-width constraint).  O^T and
           denominators accumulate in PSUM; normalization on PSUM->SBUF
           copy-out (reciprocal + partition_broadcast + tensor_mul) writes y^T
           over the dead Q^T chunk.
  phase 3: output projection, partial [T, C] per core, summed on host.

Scheduling: attention is exp-rate-bound on ScalarE (~1us/key-block vs ~0.64us
of TensorE work), so phase-1/proj matmul units are interleaved between
attention key-block units at computed ratios to keep TensorE saturated;
attention for q-chunk qc starts as soon as chunk qc's QKV is done.

Host side: per-batch pairs of partial outputs summed (2-way all-reduce of the
row-sharded Wproj) plus the rank-1 correction (bqkv_v @ Wproj + bproj), which
commutes with attention because softmax rows sum to one.  Softmax
max-subtraction skipped: scores ~N(0,1) after the 1/8 scale, exp cannot
overflow.
"""
import itertools

import numpy as np
import ml_dtypes

import concourse.bass as bass  # noqa: F401  (bass must be imported before tile)
import concourse.tile as tile
from concourse import mybir
from concourse.bacc import Bacc
from concourse.bass_utils import run_bass_kernel_spmd

F32 = mybir.dt.float32
BF16 = mybir.dt.bfloat16
BF = ml_dtypes.bfloat16

B, T, C = 4, 2048, 1024
NH = 16          # total heads
D = 64           # head dim
G = 2            # head groups (cores per batch)
HPG = NH // G    # heads per group = 8
GC = HPG * D     # columns per group = 512
CT = C // 128    # contraction tiles = 8
QCW = 512        # chunk width (phase-1 chunk == attention q-chunk)
NQC = T // QCW   # 4 q-chunks
NTT = T // 128   # 16 t-tiles
NHP = HPG // 2   # head pairs per core = 4
EXP = mybir.ActivationFunctionType.Exp

# When True, S scores land in PSUM as bf16 (ph0 in bank 0 cols [0:QCW],
# ph1 in bank 1 cols [2*QCW:3*QCW]) and exp runs as two narrowed bf16-in
# instructions per key block instead of one merged fp32-in [128,1024].
S_BF16 = False

_SENT = object()


def build(reps=1):
    nc = Bacc()
    xT = nc.dram_tensor("xT", [C, T], BF16, kind="ExternalInput")
    wqk = nc.dram_tensor("wqk", [C, 2 * GC], BF16, kind="ExternalInput")
    wv = nc.dram_tensor("wv", [C, GC], BF16, kind="ExternalInput")
    wp = nc.dram_tensor("wp", [GC, C], BF16, kind="ExternalInput")
    bqk = nc.dram_tensor("bqk", [128, 2 * GC // 128], F32, kind="ExternalInput")
    out = nc.dram_tensor("out", [T, C], F32, kind="ExternalOutput")

    with tile.TileContext(nc) as tc:
        with (
            tc.tile_pool(name="persist", bufs=1) as pp,
            tc.tile_pool(name="xc", bufs=2) as xcp,
            tc.tile_pool(name="pt", bufs=5) as ptp,
            tc.tile_pool(name="rb", bufs=3) as rbp,
            tc.tile_pool(name="ost", bufs=3) as ost,
            tc.tile_pool(name="ps", bufs=2, space="PSUM") as ps,
            tc.tile_pool(name="psS", bufs=2, space="PSUM") as psS,
            tc.tile_pool(name="psO", bufs=1, space="PSUM") as psO,
        ):
            # long-lived SBUF tensors.  QT[j][qc] doubles as y^T storage.
            # For timing builds (reps>1) two alternating sets decouple the
            # rep seams (rep r+1's QKV writes would otherwise wait on rep r's
            # late attention/proj readers, serializing the stream).
            def qkv_set(s):
                QT = [[pp.tile([128, QCW], BF16, tag=f"qt{s}_{j}_{q}",
                               name=f"qt{s}_{j}_{q}")
                       for q in range(NQC)] for j in range(NHP)]
                KT = [[pp.tile([128, QCW], BF16, tag=f"kt{s}_{j}_{q}",
                               name=f"kt{s}_{j}_{q}")
                       for q in range(NQC)] for j in range(NHP)]
                VA = [pp.tile([128, HPG, D + 1], BF16, tag=f"va{s}_{t}",
                              name=f"va{s}_{t}")
                      for t in range(NTT)]
                return QT, KT, VA

            sets = [qkv_set(0)]
            if reps > 1:
                sets.append(qkv_set(1))
            QT, KT, VA = sets[0]
            YT = QT
            WQK = [pp.tile([128, 2 * GC], BF16, tag=f"wqk{c}", name=f"wqk{c}")
                   for c in range(CT)]
            WV = [pp.tile([128, GC], BF16, tag=f"wv{c}", name=f"wv{c}")
                  for c in range(CT)]
            WP = [pp.tile([128, C], BF16, tag=f"wpr{j}", name=f"wpr{j}")
                  for j in range(GC // 128)]
            bqk_sb = pp.tile([128, 2 * GC // 128], F32)

            # weight / bias loads (DMA straight into bf16 tiles)
            for c in range(CT):
                nc.sync.dma_start(out=WQK[c], in_=wqk[128 * c:128 * (c + 1), :])
            nc.sync.dma_start(out=bqk_sb, in_=bqk[:])
            for c in range(CT):
                nc.sync.dma_start(out=WV[c], in_=wv[128 * c:128 * (c + 1), :])

            # upper-triangular (keep k<=q) bf16 mask for diagonal sub-blocks
            tri32 = pp.tile([128, 128], F32)
            nc.vector.memset(tri32, 1.0)
            nc.gpsimd.affine_select(
                out=tri32, in_=tri32, pattern=[[1, 128]],
                compare_op=mybir.AluOpType.is_ge, fill=0.0,
                base=0, channel_multiplier=-1,
            )
            tri = pp.tile([128, 128], BF16)
            nc.vector.tensor_copy(tri, tri32)
            # ones columns of VA (disjoint from the V slices written later)
            for _, _, va_s in sets:
                for t in range(NTT):
                    nc.gpsimd.memset(va_s[t][:, :, D:D + 1], 1.0)

            def p1(ch):
                XC = []
                for c in range(CT):
                    xc = xcp.tile([128, QCW], BF16, tag=f"xc{c}", name=f"xc{c}")
                    nc.sync.dma_start(
                        out=xc, in_=xT[128 * c:128 * (c + 1), QCW * ch:QCW * (ch + 1)]
                    )
                    XC.append(xc)
                yield
                # Q^T / K^T row-tiles (m<4 -> Q pair-tile m, m>=4 -> K pair m-4)
                # split into half-units (4 MMs each) so interleave filler is
                # ~0.55us-granular; the open accumulation group across the
                # yield is fine (other tiles' matmuls don't touch this bank)
                for m in range(2 * GC // 128):
                    acc = ps.tile([128, QCW], F32, tag="pp", name="pp")
                    for c in range(CT // 2):
                        nc.tensor.matmul(
                            acc, WQK[c][:, 128 * m:128 * (m + 1)], XC[c],
                            start=(c == 0), stop=False,
                        )
                    yield
                    for c in range(CT // 2, CT):
                        nc.tensor.matmul(
                            acc, WQK[c][:, 128 * m:128 * (m + 1)], XC[c],
                            start=False, stop=(c == CT - 1),
                        )
                    dst = QT[m][ch] if m < NHP else KT[m - NHP][ch]
                    nc.vector.tensor_scalar_add(dst, acc, bqk_sb[:, m:m + 1])
                    yield
                for ti in range(QCW // 128):
                    t = (QCW // 128) * ch + ti
                    acc = ps.tile([128, QCW], F32, tag="pp", name="pp")
                    for c in range(CT // 2):
                        nc.tensor.matmul(
                            acc, XC[c][:, 128 * ti:128 * (ti + 1)], WV[c],
                            start=(c == 0), stop=False,
                        )
                    yield
                    for c in range(CT // 2, CT):
                        nc.tensor.matmul(
                            acc, XC[c][:, 128 * ti:128 * (ti + 1)], WV[c],
                            start=False, stop=(c == CT - 1),
                        )
                    nc.vector.tensor_copy(
                        VA[t][:, :, 0:D], acc.rearrange("p (h d) -> p h d", h=HPG)
                    )
                    yield

            N_P1 = 1 + 16 + 8

            def att(qc):
                # Software-pipelined: S/exp/mask for kb+1 are emitted BEFORE
                # AV for kb, and the interleave filler (yield) lands between
                # them — so TensorE's in-order queue covers the exp latency
                # with filler matmuls instead of blocking on AV, and ScalarE's
                # exp stream (fed by the already-issued S) never starves.
                kbmax = 4 * (qc + 1)

                def s_exp(hp, kb):
                    j = kb - 4 * qc
                    s_off = 128 * j if j > 0 else 0
                    P = ptp.tile([128, 2 * QCW], BF16, tag="p", name="p")
                    if S_BF16:
                        # ph0 -> bank 0, ph1 -> bank 1 (keeps the concurrent
                        # 0/64 row-group pair off the same PSUM bank)
                        S = psS.tile([128, 4 * QCW], BF16, tag="s", name="s")
                        for ph in range(2):
                            p_sl = slice(64 * ph, 64 * (ph + 1))
                            nc.tensor.matmul(
                                S[:, 2 * QCW * ph + s_off:2 * QCW * ph + QCW],
                                KT[hp][kb // 4][p_sl, 128 * (kb % 4):128 * (kb % 4 + 1)],
                                QT[hp][qc][p_sl, s_off:],
                                start=True, stop=True,
                            )
                        for ph in range(2):
                            nc.scalar.activation(
                                out=P[:, QCW * ph + s_off:QCW * (ph + 1)],
                                in_=S[:, 2 * QCW * ph + s_off:2 * QCW * ph + QCW],
                                func=EXP, scale=0.125,
                            )
                    else:
                        S = psS.tile([128, 2 * QCW], F32, tag="s", name="s")
                        for ph in range(2):
                            p_sl = slice(64 * ph, 64 * (ph + 1))
                            nc.tensor.matmul(
                                S[:, QCW * ph + s_off:QCW * (ph + 1)],
                                KT[hp][kb // 4][p_sl, 128 * (kb % 4):128 * (kb % 4 + 1)],
                                QT[hp][qc][p_sl, s_off:],
                                start=True, stop=True,
                            )
                        nc.scalar.activation(out=P, in_=S, func=EXP, scale=0.125)
                    if j >= 0:
                        for ph in range(2):
                            off = QCW * ph + 128 * j
                            nc.vector.tensor_mul(
                                P[:, off:off + 128], P[:, off:off + 128], tri
                            )
                    return P, s_off

                def norm(hp, O):
                    rc = rbp.tile([1, 2 * QCW], F32, tag="rc", name="rc", bufs=1)
                    nc.vector.reciprocal(rc, O[D:D + 1, :])
                    for ph in range(2):
                        rb = rbp.tile([64, QCW], F32, tag="rb", name="rb")
                        nc.gpsimd.partition_broadcast(rb, rc[0:1, QCW * ph:QCW * (ph + 1)])
                        nc.vector.tensor_mul(
                            YT[hp][qc][64 * ph:64 * (ph + 1), :],
                            O[0:D, QCW * ph:QCW * (ph + 1)], rb,
                        )

                pend = None  # previous head-pair's (hp, O) awaiting norm
                for hp in range(NHP):
                    O = psO.tile([D + 1, 2 * QCW], F32, tag="o", name="o")
                    pipe = s_exp(hp, 0)
                    if pend is not None:
                        # psO bufs=1: norm(hp-1) MUST be emitted before any
                        # AV of this hp (same PSUM addresses)
                        norm(*pend)
                        pend = None
                        yield
                    for kb in range(kbmax):
                        P, s_off = pipe
                        if kb + 1 < kbmax:
                            pipe = s_exp(hp, kb + 1)
                        yield
                        for ph in range(2):
                            nc.tensor.matmul(
                                O[:, QCW * ph + s_off:QCW * (ph + 1)],
                                VA[kb][:, 2 * hp + ph, :],
                                P[:, QCW * ph + s_off:QCW * (ph + 1)],
                                start=(kb == 0), stop=(kb == kbmax - 1),
                            )
                    pend = (hp, O)
                norm(*pend)
                yield

            def n_att(qc):
                return NHP * (4 * (qc + 1)) + NHP

            def proj(qc):
                for ti in range(4):
                    t = 4 * qc + ti
                    for nn in range(2):
                        acc = ps.tile([128, QCW], F32, tag="pp", name="pp")
                        for jj in range(GC // 128):
                            nc.tensor.matmul(
                                acc,
                                YT[jj][qc][:, 128 * ti:128 * (ti + 1)],
                                WP[jj][:, QCW * nn:QCW * (nn + 1)],
                                start=(jj == 0), stop=(jj == GC // 128 - 1),
                            )
                        o = ost.tile([128, QCW], F32, tag="o", name="o")
                        nc.vector.tensor_copy(o, acc)
                        nc.sync.dma_start(
                            out=out[128 * t:128 * (t + 1), QCW * nn:QCW * (nn + 1)],
                            in_=o,
                        )
                        yield

            def load_wp():
                for jx in range(GC // 128):
                    nc.sync.dma_start(out=WP[jx], in_=wp[128 * jx:128 * (jx + 1), :])
                yield

            def interleave(main_gen, n_main, fill_gen, n_fill):
                i = pulled = 0
                for _ in main_gen:
                    i += 1
                    tgt = i * n_fill // n_main
                    while pulled < tgt and next(fill_gen, _SENT) is not _SENT:
                        pulled += 1
                while next(fill_gen, _SENT) is not _SENT:
                    pass

            for rep in range(reps):
                QT, KT, VA = sets[rep % len(sets)]
                YT = QT
                for _ in p1(0):
                    pass
                interleave(att(0), n_att(0), p1(1), N_P1)
                fill1 = itertools.chain(p1(2), load_wp()) if rep == 0 else p1(2)
                interleave(att(1), n_att(1), fill1, N_P1 + 1)
                interleave(att(2), n_att(2), p1(3), N_P1)
                interleave(
                    att(3), n_att(3),
                    itertools.chain(proj(0), proj(1), proj(2)), 24,
                )
                for _ in proj(3):
                    pass
    nc.finalize()
    return nc


_NC = None


def _get_nc():
    global _NC
    if _NC is None:
        _NC = build()
    return _NC


def _shard(x, Wqkv, bqkv, Wproj):
    in_maps = []
    for core in range(8):
        b, g = core // G, core % G
        cs = slice(GC * g, GC * (g + 1))
        wqk_h = np.concatenate([Wqkv[:, cs], Wqkv[:, C:][:, cs]], axis=1)
        bqk_h = np.concatenate([bqkv[cs], bqkv[C:][cs.start:cs.stop]])
        in_maps.append({
            "xT": np.ascontiguousarray(x[b].T.astype(BF)),
            "wqk": np.ascontiguousarray(wqk_h.astype(BF)),
            "wv": np.ascontiguousarray(Wqkv[:, 2 * C:][:, cs].astype(BF)),
            "wp": np.ascontiguousarray(Wproj[cs, :].astype(BF)),
            "bqk": np.ascontiguousarray(bqk_h.reshape(2 * GC // 128, 128).T),
        })
    return in_maps


def kernel(x, Wqkv, bqkv, Wproj, bproj, _want_results=False, **run_kwargs):
    x = np.ascontiguousarray(np.asarray(x, dtype=np.float32))
    Wqkv = np.ascontiguousarray(np.asarray(Wqkv, dtype=np.float32))
    bqkv = np.ascontiguousarray(np.asarray(bqkv, dtype=np.float32))
    Wproj = np.ascontiguousarray(np.asarray(Wproj, dtype=np.float32))
    bproj = np.ascontiguousarray(np.asarray(bproj, dtype=np.float32))

    nc = _get_nc()
    in_maps = _shard(x, Wqkv, bqkv, Wproj)
    res = run_bass_kernel_spmd(nc, in_maps, core_ids=list(range(8)), **run_kwargs)

    out = np.empty((B, T, C), dtype=np.float32)
    for b in range(B):
        out[b] = res.results[G * b]["out"]
        for g in range(1, G):
            out[b] += res.results[G * b + g]["out"]
    # rank-1 corrections: v-bias (rows of softmax sum to 1) and proj bias
    out += bqkv[2 * C:] @ Wproj + bproj
    if _want_results:
        return out, res
    return out
